# revision 19
# baseline (speedup 1.0000x reference)
"""Trainium2 Bass kernel for a dense transformer block (B=2, S=1024, D=1024,
H=16, KV-cache P=1024), SPMD over 8 NeuronCores.

Sharding: 8-way split of the (b, s) query rows — cores 0-3 handle batch 0,
cores 4-7 batch 1, each owning 256 rows. The k/v projection (needed for all
rows of the batch by every query) is replicated inside each 4-core batch
group, which removes every cross-core collective. Per-core inputs are
column-permuted so the owned rows always sit in block 0 — one compiled
program serves all 8 cores.

Activations live transposed ([D, S]) on chip so weight matrices feed the
TensorEngine in their natural layout and biases become per-partition vectors.
LayerNorm scale/shift and all projection biases are folded host-side into the
weights/biases. Matmuls run in bf16 (fp32 accumulate); softmax uses
exp(logit - 12) with the denominator produced by an extra ones-column in the
AV matmul's stationary operand.
"""

import numpy as np
import ml_dtypes

BF16 = ml_dtypes.bfloat16

B, S, P, D, H, DH = 2, 1024, 1024, 1024, 16, 64
T = P + S          # total attended keys
F = 4 * D          # MLP hidden
RB = S // 4        # rows owned per core
EPS = 1e-12
ESUB = 12.0        # constant subtracted inside exp (replaces max-subtraction)

_CACHE = {}


def _build_nc(dbg=False):
    import concourse.mybir as mybir
    import concourse.tile as tile
    from concourse import bacc
    from concourse.masks import make_identity

    F32, BF = mybir.dt.float32, mybir.dt.bfloat16
    AF = mybir.ActivationFunctionType
    ADD = mybir.AluOpType.add
    MULT = mybir.AluOpType.mult
    SUBT = mybir.AluOpType.subtract

    nc = bacc.Bacc(None, target_bir_lowering=False, num_devices=8)

    def par(name, shape, dt, out=False):
        return nc.declare_dram_parameter(name, list(shape), dt, isOutput=out)

    xTb = par("xTb", [D, S], BF)            # x[b]^T (permuted cols), bf16
    xTo = par("xTo", [D, RB], F32)          # x[b]^T own cols, f32
    wq = par("wq", [D, D], BF)
    wk = par("wk", [D, D], BF)
    wv = par("wv", [D, D], BF)
    w1 = par("w1", [1, 3 * D], BF)          # column sums of folded wq|wk|wv
    bq = par("bq", [128, 8], F32)           # [p, m] = bias[m*128+p]
    bk = par("bk", [128, 8], F32)
    bvT = par("bvT", [1, D], F32)           # v bias as a row (free-dim layout)
    wp = par("wp", [D, D], BF)
    bp = par("bp", [128, 8], F32)
    wfc = par("wfc", [32, 128, 8, 128], BF)  # [m, p, k, c] tiles
    bfc = par("bfc", [128, 32], F32)
    wfc2 = par("wfc2", [F, D], BF)
    bfc2T = par("bfc2T", [1, D], F32)
    kc = par("kc", [8, 128, P], BF)         # k cache, head pairs packed
    vc = par("vc", [H, 128, 8, 65], BF)     # v cache + ones col, per head
    xo = par("xo", [RB, D], F32, out=True)
    if dbg:
        d_aT = par("d_aT", [128, 8, RB], F32, out=True)
        d_xmid = par("d_xmid", [128, 8, RB], F32, out=True)
        d_h2T = par("d_h2T", [128, 8, RB], F32, out=True)
        d_mT = par("d_mT", [128, 32, RB], F32, out=True)
        d_pav = par("d_pav", [H, 65, RB], F32, out=True)
    nk = par("nk", [D, RB], F32, out=True)
    nv = par("nv", [RB, D], F32, out=True)

    with tile.TileContext(nc) as tc:
        from contextlib import ExitStack

        with (
            tc.tile_pool(name="const", bufs=1) as const,
            tc.tile_pool(name="pers", bufs=1) as pers,
            tc.tile_pool(name="wrow", bufs=8) as wrow,
            tc.tile_pool(name="wfcrow", bufs=2) as wfcrow,
            tc.tile_pool(name="work", bufs=2) as work,
            tc.tile_pool(name="stats", bufs=1) as statsp,
            tc.tile_pool(name="outp", bufs=2) as outp,
            tc.tile_pool(name="pbig", bufs=2, space="PSUM") as pbig,
            tc.tile_pool(name="psmall", bufs=4, space="PSUM") as psmall,
        ):
            # ---- constants ----
            onesb = const.tile([128, 1], BF, tag="onesb", name="onesb")
            nc.vector.memset(onesb[:], 1.0)
            identf = const.tile([128, 128], F32, tag="ident", name="ident")
            make_identity(nc, identf[:])
            identb = const.tile([128, 128], BF, tag="identb", name="identb")
            make_identity(nc, identb[:])
            # cvec columns: 0 nbias(-ESUB), 1 eps, 2:10 bq, 10:18 bk, 18:26 bp,
            # 26:58 bfc
            cvec = const.tile([128, 58], F32, tag="cvec", name="cvec")
            nc.vector.memset(cvec[:, 0:1], -ESUB)
            nc.vector.memset(cvec[:, 1:2], EPS)
            nc.sync.dma_start(out=cvec[:, 2:10], in_=bq[:])
            nc.sync.dma_start(out=cvec[:, 10:18], in_=bk[:])
            nc.sync.dma_start(out=cvec[:, 18:26], in_=bp[:])
            nc.sync.dma_start(out=cvec[:, 26:58], in_=bfc[:])
            nbias = cvec[:, 0:1]
            epst = cvec[:, 1:2]
            bqt = cvec[:, 2:10]
            bkt = cvec[:, 10:18]
            bpt = cvec[:, 18:26]
            bfct = cvec[:, 26:58]

            bvb = const.tile([128, D], F32, tag="bvb", name="bvb")
            bfc2b = const.tile([128, D], F32, tag="bfc2b", name="bfc2b")

            # ---- persistent activations (consolidated 3D tiles) ----
            xTown = pers.tile([128, 8, RB], F32, tag="xTown", name="xTown")
            qT = pers.tile([128, 8, RB], BF, tag="qT", name="qT")
            aT = pers.tile([128, 8, RB], BF, tag="aT", name="aT")
            xmid = pers.tile([128, 8, RB], F32, tag="xmid", name="xmid")
            xmb = pers.tile([128, 8, RB], BF, tag="xmb", name="xmb")
            h2T = pers.tile([128, 8, RB], BF, tag="h2T", name="h2T")
            mT = pers.tile([128, 32, RB], BF, tag="mT", name="mT")
            xmn = pers.tile([128, 2, D], F32, tag="xmn", name="xmn")  # x_mid natural

            st_a = statsp.tile([1, S], F32, tag="st_a", name="st_a")
            st_b = statsp.tile([1, S], F32, tag="st_b", name="st_b")
            st_c = statsp.tile([1, S], F32, tag="st_c", name="st_c")
            st_d = statsp.tile([1, S], F32, tag="st_d", name="st_d")
            st_rb = statsp.tile([1, S], BF, tag="st_rb", name="st_rb")
            st_sb = statsp.tile([1, S], BF, tag="st_sb", name="st_sb")

            def ln_stats(ps_sx, ps_sx2, n, nfree):
                """returns (rstd_b, negmur_b) [1, nfree] bf16 slices."""
                mean = st_a[:, 0:nfree]
                var = st_b[:, 0:nfree]
                msq = st_c[:, 0:nfree]
                std = st_d[:, 0:nfree]
                rstd = st_c[:, 0:nfree]   # reuses msq slot
                nmr = st_a[:, 0:nfree]    # reuses mean slot
                nc.vector.tensor_scalar_mul(mean, ps_sx[:], 1.0 / n)
                nc.vector.tensor_scalar_mul(var, ps_sx2[:], 1.0 / n)
                nc.vector.tensor_tensor(msq, mean, mean, op=MULT)
                nc.vector.tensor_tensor(var, var, msq, op=SUBT)
                nc.scalar.activation(std, var, AF.Sqrt, bias=epst[0:1, :])
                nc.vector.reciprocal_approx_fast(out=rstd, in_=std)
                nc.vector.tensor_tensor(nmr, mean, rstd, op=MULT)
                nc.vector.tensor_scalar_mul(nmr, nmr, -1.0)
                nc.vector.tensor_copy(st_rb[:, 0:nfree], rstd)
                nc.vector.tensor_copy(st_sb[:, 0:nfree], nmr)
                return st_rb[:, 0:nfree], st_sb[:, 0:nfree]

            scope_c_cm = tc.tile_pool(name="scope_c", bufs=1)
            scope_c = scope_c_cm.__enter__()
            kT = scope_c.tile([128, 8, S], BF, tag="kT", name="kT")
            vS = scope_c.tile([128, 8, D], BF, tag="vS", name="vS")
            scope_b_cm = tc.tile_pool(name="scope_b", bufs=1)
            scope_b = scope_b_cm.__enter__()
            if True:
                xT = scope_b.tile([128, 8, S], BF, tag="xT", name="xT")
                hT = xT  # LayerNorm applied in place
                for i in range(8):
                    nc.sync.dma_start(
                        out=xT[:, i, :], in_=xTb[i * 128 : (i + 1) * 128, :]
                    )

                # ===== LN1 over full S =====
                ps_sx = pbig.tile([1, S], F32, tag="big", name="ps_sx")
                ps_sx2 = pbig.tile([1, S], F32, tag="big", name="ps_sx2")
                for i in range(8):
                    x2t = work.tile([128, S], BF, tag="x2", name="x2t")
                    nc.vector.tensor_tensor(x2t[:], xT[:, i, :], xT[:, i, :], op=MULT)
                    for j in range(2):
                        sl = slice(j * 512, (j + 1) * 512)
                        nc.tensor.matmul(
                            ps_sx[:, sl], onesb[:], xT[:, i, sl],
                            start=(i == 0), stop=(i == 7),
                        )
                        nc.tensor.matmul(
                            ps_sx2[:, sl], onesb[:], x2t[:, sl],
                            start=(i == 0), stop=(i == 7),
                        )
                # LN1 folded form: r (rstd) + m-tilde rows; no apply pass
                mean1 = st_a[:, 0:S]
                var1 = st_b[:, 0:S]
                msq1 = st_c[:, 0:S]
                std1 = st_d[:, 0:S]
                nc.vector.tensor_scalar_mul(mean1, ps_sx[:], 1.0 / D)
                nc.vector.tensor_scalar_mul(var1, ps_sx2[:], 1.0 / D)
                nc.vector.tensor_tensor(msq1, mean1, mean1, op=MULT)
                nc.vector.tensor_tensor(var1, var1, msq1, op=SUBT)
                nc.scalar.activation(std1, var1, AF.Sqrt, bias=epst[0:1, :])
                nc.vector.reciprocal_approx_fast(out=msq1, in_=std1)
                nc.vector.tensor_copy(st_rb[:, 0:S], msq1)          # r bf16
                nc.vector.tensor_scalar_mul(mean1, mean1, -1.0)
                nc.vector.tensor_copy(st_sb[:, 0:S], mean1)         # m-tilde bf16
                xaug = st_sb
                r1bb = scope_b.tile([128, S], BF, tag="r1bb", name="r1bb")
                nc.gpsimd.partition_broadcast(r1bb[:], st_rb[:, 0:S])
                rcol = scope_b.tile([128, 8], F32, tag="rcol", name="rcol")
                w1r = scope_b.tile([1, 3 * D], BF, tag="w1r", name="w1r")
                nc.sync.dma_start(out=w1r[:], in_=w1[:])

            # ===== q^T [D, RB] (own rows) =====
            wrows = [None] * 8
            for kk in range(8):
                wrows[kk] = wrow.tile([128, D], BF, tag="wrow", name="wqr")
                nc.gpsimd.dma_start(
                    out=wrows[kk][:], in_=wq[kk * 128 : (kk + 1) * 128, :]
                )
            for m in range(8):
                pq = psmall.tile([128, RB], F32, tag="small", name="pq")
                for kk in range(8):
                    nc.tensor.matmul(
                        pq[:], wrows[kk][:, m * 128 : (m + 1) * 128],
                        xT[:, kk, 0:RB], start=(kk == 0), stop=False,
                    )
                nc.tensor.matmul(
                    pq[:], w1r[:, m * 128 : (m + 1) * 128],
                    xaug[:, 0:RB], start=False, stop=True,
                )
                nc.vector.tensor_tensor(
                    qT[:, m, :], pq[:], r1bb[:, 0:RB], op=MULT
                )
                nc.vector.tensor_scalar_add(
                    qT[:, m, :], qT[:, m, :], bqt[:, m : m + 1]
                )

            # ===== k^T [D, S] (all rows; new_k out) =====
            for kk in range(8):
                wrows[kk] = wrow.tile([128, D], BF, tag="wrow", name="wkr")
                nc.scalar.dma_start(
                    out=wrows[kk][:], in_=wk[kk * 128 : (kk + 1) * 128, :]
                )
            for m in range(8):
                for j in range(2):
                    sl = slice(j * 512, (j + 1) * 512)
                    pk = psmall.tile([128, 512], F32, tag="small", name="pk")
                    for kk in range(8):
                        nc.tensor.matmul(
                            pk[:], wrows[kk][:, m * 128 : (m + 1) * 128],
                            xT[:, kk, sl], start=(kk == 0), stop=False,
                        )
                    nc.tensor.matmul(
                        pk[:], w1r[:, D + m * 128 : D + (m + 1) * 128],
                        xaug[:, sl], start=False, stop=True,
                    )
                    nc.vector.tensor_tensor(
                        kT[:, m, sl], pk[:], r1bb[:, sl], op=MULT
                    )
                    nc.vector.tensor_scalar_add(
                        kT[:, m, sl], kT[:, m, sl], bkt[:, m : m + 1]
                    )
                    if j == 0:
                        nkf = outp.tile([128, 512], F32, tag="of", name="nkf")
                        nc.vector.tensor_tensor(
                            nkf[:, 0:RB], pk[:, 0:RB], r1bb[:, 0:RB], op=MULT
                        )
                        nc.vector.tensor_scalar_add(
                            nkf[:, 0:RB], nkf[:, 0:RB], bkt[:, m : m + 1]
                        )
                        nc.sync.dma_start(
                            out=nk[m * 128 : (m + 1) * 128, :], in_=nkf[:, 0:RB]
                        )

            # ===== v [S, D] natural (all rows; new_v out) =====
            for kk in range(8):
                wrows[kk] = wrow.tile([128, D], BF, tag="wrow", name="wvr")
                nc.scalar.dma_start(
                    out=wrows[kk][:], in_=wv[kk * 128 : (kk + 1) * 128, :]
                )
            nc.scalar.dma_start(out=bvb[:], in_=bvT[:].to_broadcast((128, D)))
            for m in range(8):
                ptb = psmall.tile([128, 128], BF, tag="small", name="ptb")
                nc.tensor.transpose(
                    ptb[:], r1bb[:, m * 128 : (m + 1) * 128], identb[:]
                )
                nc.vector.tensor_copy(rcol[:, m : m + 1], ptb[:, 0:1])
            for m in range(8):
                for j in range(2):
                    sl = slice(j * 512, (j + 1) * 512)
                    pv = psmall.tile([128, 512], F32, tag="small", name="pv")
                    for kk in range(8):
                        nc.tensor.matmul(
                            pv[:], xT[:, kk, m * 128 : (m + 1) * 128],
                            wrows[kk][:, sl], start=(kk == 0), stop=False,
                        )
                    nc.tensor.matmul(
                        pv[:], xaug[:, m * 128 : (m + 1) * 128],
                        w1r[:, 2 * D + j * 512 : 2 * D + (j + 1) * 512],
                        start=False, stop=True,
                    )
                    nc.vector.tensor_scalar_mul(
                        vS[:, m, sl], pv[:], rcol[:, m : m + 1]
                    )
                    nc.vector.tensor_tensor(
                        vS[:, m, sl], vS[:, m, sl], bvb[:, sl], op=ADD
                    )
                    if m < 2:
                        nvf = outp.tile([128, 512], F32, tag="of", name="nvf")
                        nc.vector.tensor_scalar_mul(
                            nvf[:], pv[:], rcol[:, m : m + 1]
                        )
                        nc.vector.tensor_tensor(nvf[:], nvf[:], bvb[:, sl], op=ADD)
                        nc.sync.dma_start(
                            out=nv[m * 128 : (m + 1) * 128, sl], in_=nvf[:]
                        )
            scope_b_cm.__exit__(None, None, None)  # xT freed

            nc.gpsimd.dma_start(
                out=xTown[:], in_=xTo[:].rearrange("(i p) s -> p i s", p=128)
            )
            # ===== attention (16 heads) =====
            kch = [None]
            for h in range(H):
                hp, hr = h // 2, (h % 2) * 64
                if h % 2 == 0:
                    kch[0] = scope_c.tile(
                        [128, P], BF, tag="kch", name="kch", bufs=2
                    )
                    nc.gpsimd.dma_start(out=kch[0][:], in_=kc[hp])
                kcht = kch[0]
                va = scope_c.tile([128, 8, 65], BF, tag="va", name="va", bufs=2)
                nc.vector.tensor_copy(
                    va[:, :, 0:64], vS[:, :, h * 64 : (h + 1) * 64]
                )
                nc.vector.memset(va[:, :, 64:65], 1.0)
                vct = scope_c.tile([128, 8, 65], BF, tag="vct", name="vct", bufs=2)
                nc.gpsimd.dma_start(out=vct[:], in_=vc[h])

                pav = psmall.tile([65, RB], F32, tag="small", name="pav")
                for g in range(4):  # 4 groups of 4 t-tiles
                    pl = pbig.tile([128, 4 * RB], F32, tag="big", name="pl")
                    for tl in range(4):
                        tt = g * 4 + tl
                        if tt < 8:
                            lh = kcht[hr : hr + 64, tt * 128 : (tt + 1) * 128]
                        else:
                            lh = kT[hr : hr + 64, hp, (tt - 8) * 128 : (tt - 7) * 128]
                        nc.tensor.matmul(
                            pl[:, tl * RB : (tl + 1) * RB], lh,
                            qT[hr : hr + 64, hp, :], start=True, stop=True,
                        )
                    wex = scope_c.tile(
                        [128, 4 * RB], BF, tag="wex", name="wex", bufs=2
                    )
                    nc.scalar.activation(wex[:], pl[:], AF.Exp, bias=nbias)
                    for tl in range(4):
                        tt = g * 4 + tl
                        vv = vct[:, tt, :] if tt < 8 else va[:, tt - 8, :]
                        nc.tensor.matmul(
                            pav[:], vv, wex[:, tl * RB : (tl + 1) * RB],
                            start=(tt == 0), stop=(tt == 15),
                        )
                if dbg:
                    dpav = outp.tile([65, RB], F32, tag="of", name="dbgpav")
                    nc.vector.tensor_copy(dpav[:], pav[:])
                    nc.sync.dma_start(out=d_pav[h], in_=dpav[:])
                rec = scope_c.tile([1, 2, RB], F32, tag="rec", name="rec", bufs=1)
                nc.vector.tensor_copy(rec[:, 0, :], pav[64:65, :])
                nc.vector.reciprocal_approx_fast(out=rec[:, 1, :], in_=rec[:, 0, :])
                rb64 = scope_c.tile([64, RB], F32, tag="rb64", name="rb64", bufs=2)
                nc.gpsimd.partition_broadcast(rb64[:], rec[:, 1, :])
                nc.vector.tensor_tensor(
                    aT[hr : hr + 64, hp, :], pav[0:64, :], rb64[:], op=MULT
                )

            if dbg:
                for i in range(8):
                    dt_ = outp.tile([128, RB], F32, tag="of", name="dbg1")
                    nc.vector.tensor_copy(dt_[:], aT[:, i, :])
                    nc.sync.dma_start(out=d_aT[:, i, :], in_=dt_[:])

            # ===== proj + residual -> x_mid^T [D, RB] f32 =====
            for kk in range(8):
                wrows[kk] = wrow.tile([128, D], BF, tag="wrow", name="wpr")
                nc.scalar.dma_start(
                    out=wrows[kk][:], in_=wp[kk * 128 : (kk + 1) * 128, :]
                )
            for m in range(8):
                pp = psmall.tile([128, RB], F32, tag="small", name="pp")
                for kk in range(8):
                    nc.tensor.matmul(
                        pp[:], wrows[kk][:, m * 128 : (m + 1) * 128],
                        aT[:, kk, :], start=(kk == 0), stop=(kk == 7),
                    )
                nc.vector.tensor_scalar_add(xmid[:, m, :], pp[:], bpt[:, m : m + 1])
                nc.vector.tensor_tensor(
                    xmid[:, m, :], xmid[:, m, :], xTown[:, m, :], op=ADD
                )
            scope_c_cm.__exit__(None, None, None)  # kT / vS / attn work freed

            if dbg:
                for i in range(8):
                    dt_ = outp.tile([128, RB], F32, tag="of", name="dbg2")
                    nc.vector.tensor_copy(dt_[:], xmid[:, i, :])
                    nc.sync.dma_start(out=d_xmid[:, i, :], in_=dt_[:])

            # ===== LN2 (over own RB cols) =====
            ps2_sx = pbig.tile([1, RB], F32, tag="big", name="ps2_sx")
            ps2_sx2 = pbig.tile([1, RB], F32, tag="big", name="ps2_sx2")
            for i in range(8):
                nc.vector.tensor_copy(xmb[:, i, :], xmid[:, i, :])
                x2t = work.tile([128, RB], BF, tag="x2", name="x2s")
                nc.vector.tensor_tensor(x2t[:], xmb[:, i, :], xmb[:, i, :], op=MULT)
                nc.tensor.matmul(
                    ps2_sx[:], onesb[:], xmb[:, i, :], start=(i == 0), stop=(i == 7)
                )
                nc.tensor.matmul(
                    ps2_sx2[:], onesb[:], x2t[:], start=(i == 0), stop=(i == 7)
                )
            r2b, s2b = ln_stats(ps2_sx, ps2_sx2, D, RB)
            r2bb = work.tile([128, RB], BF, tag="r2bb", name="r2bb")
            nc.gpsimd.partition_broadcast(r2bb[:], r2b)
            s2bb = work.tile([128, RB], BF, tag="s2bb", name="s2bb")
            nc.gpsimd.partition_broadcast(s2bb[:], s2b)
            for i in range(8):
                nc.vector.tensor_tensor(h2T[:, i, :], xmb[:, i, :], r2bb[:], op=MULT)
                nc.vector.tensor_tensor(h2T[:, i, :], h2T[:, i, :], s2bb[:], op=ADD)

            if dbg:
                for i in range(8):
                    dt_ = outp.tile([128, RB], F32, tag="of", name="dbg3")
                    nc.vector.tensor_copy(dt_[:], h2T[:, i, :])
                    nc.sync.dma_start(out=d_h2T[:, i, :], in_=dt_[:])

            # ===== transpose x_mid -> natural [RB, D] (for final residual) ==
            for m in range(8):
                for j in range(2):
                    ptr = pbig.tile([128, 128], F32, tag="big", name="ptr")
                    nc.tensor.transpose(
                        ptr[:], xmid[:, m, j * 128 : (j + 1) * 128], identf[:]
                    )
                    nc.vector.tensor_copy(
                        xmn[:, j, m * 128 : (m + 1) * 128], ptr[:]
                    )

            # ===== MLP fc + gelu -> m^T [F, RB] bf16 =====
            for m in range(32):
                wfcm = wfcrow.tile([128, 8, 128], BF, tag="wfcm", name="wfcm", bufs=6)
                nc.sync.dma_start(out=wfcm[:], in_=wfc[m])  # [p, k, c] both sides
                pf = psmall.tile([128, RB], F32, tag="small", name="pf")
                for kk in range(8):
                    nc.tensor.matmul(
                        pf[:], wfcm[:, kk, :],
                        h2T[:, kk, :], start=(kk == 0), stop=(kk == 7),
                    )
                nc.scalar.activation(
                    mT[:, m, :], pf[:], AF.Gelu, bias=bfct[:, m : m + 1]
                )

            if dbg:
                for i in range(32):
                    dt_ = outp.tile([128, RB], F32, tag="of", name="dbg4")
                    nc.vector.tensor_copy(dt_[:], mT[:, i, :])
                    nc.sync.dma_start(out=d_mT[:, i, :], in_=dt_[:])

            nc.scalar.dma_start(
                out=bfc2b[:], in_=bfc2T[:].to_broadcast((128, D))
            )
            scope_d_cm = tc.tile_pool(name="scope_d", bufs=1)
            scope_d = scope_d_cm.__enter__()
            # ===== MLP fc2 (natural orientation) + residual -> x_out ========
            pf2 = [
                pbig.tile([128, D], F32, tag="big", name="pf2a"),
                pbig.tile([128, D], F32, tag="big", name="pf2b"),
            ]
            for kk in range(32):
                wr2 = scope_d.tile([128, D], BF, tag="wfc2r", name="wfc2r", bufs=16)
                eng = nc.sync if kk % 2 == 0 else nc.scalar
                eng.dma_start(
                    out=wr2[:], in_=wfc2[kk * 128 : (kk + 1) * 128, :]
                )
                for m in range(2):
                    for j in range(2):
                        sl = slice(j * 512, (j + 1) * 512)
                        nc.tensor.matmul(
                            pf2[m][:, sl],
                            mT[:, kk, m * 128 : (m + 1) * 128],
                            wr2[:, sl], start=(kk == 0), stop=(kk == 31),
                        )
            for m in range(2):
                for j in range(2):
                    sl = slice(j * 512, (j + 1) * 512)
                    xon = outp.tile([128, 512], F32, tag="of", name="xon")
                    nc.vector.tensor_tensor(
                        xon[:], pf2[m][:, sl], bfc2b[:, sl], op=ADD
                    )
                    nc.vector.tensor_tensor(
                        xon[:], xon[:], xmn[:, m, sl], op=ADD
                    )
                    nc.sync.dma_start(
                        out=xo[m * 128 : (m + 1) * 128, sl], in_=xon[:]
                    )
            scope_d_cm.__exit__(None, None, None)

    nc.compile()
    return nc


def _prep_inputs(inputs):
    x = np.asarray(inputs["x"], np.float32)
    k = np.asarray(inputs["k"], np.float32)
    v = np.asarray(inputs["v"], np.float32)
    ln1_w = np.asarray(inputs["ln1_w"], np.float32)
    ln1_b = np.asarray(inputs["ln1_b"], np.float32)
    ln2_w = np.asarray(inputs["ln2_w"], np.float32)
    ln2_b = np.asarray(inputs["ln2_b"], np.float32)
    w_attn = np.asarray(inputs["w_attn"], np.float32)
    b_attn = np.asarray(inputs["b_attn"], np.float32)
    w_proj = np.asarray(inputs["w_proj"], np.float32)
    b_proj = np.asarray(inputs["b_proj"], np.float32)
    w_fc = np.asarray(inputs["w_fc"], np.float32)
    b_fc = np.asarray(inputs["b_fc"], np.float32)
    w_fc2 = np.asarray(inputs["w_fc2"], np.float32)
    b_fc2 = np.asarray(inputs["b_fc2"], np.float32)

    W1 = ln1_w[:, None] * w_attn
    bqkv = b_attn + ln1_b @ w_attn
    Wfc = ln2_w[:, None] * w_fc
    bfc_f = b_fc + ln2_b @ w_fc

    def colmaj(b):  # [n*128] -> [128, n] with [p, m] = b[m*128+p]
        return np.ascontiguousarray(b.reshape(-1, 128).T, dtype=np.float32)

    shared = {
        "wq": np.ascontiguousarray(W1[:, :D]).astype(BF16),
        "wk": np.ascontiguousarray(W1[:, D : 2 * D]).astype(BF16),
        "wv": np.ascontiguousarray(W1[:, 2 * D :]).astype(BF16),
        "w1": np.ascontiguousarray(
            np.concatenate(
                [W1[:, :D].sum(0), W1[:, D : 2 * D].sum(0), W1[:, 2 * D :].sum(0)]
            ).reshape(1, 3 * D)
        ).astype(BF16),
        "bq": colmaj(bqkv[:D]),
        "bk": colmaj(bqkv[D : 2 * D]),
        "bvT": np.ascontiguousarray(bqkv[2 * D :].reshape(1, D), dtype=np.float32),
        "wp": w_proj.astype(BF16),
        "bp": colmaj(b_proj),
        "wfc": np.ascontiguousarray(
            Wfc.reshape(8, 128, 32, 128).transpose(2, 1, 0, 3)
        ).astype(BF16),
        "bfc": colmaj(bfc_f),
        "wfc2": w_fc2.astype(BF16),
        "bfc2T": np.ascontiguousarray(b_fc2.reshape(1, D), dtype=np.float32),
    }

    in_maps = []
    for c in range(8):
        b, r = c // 4, c % 4
        own = np.arange(r * RB, (r + 1) * RB)
        rest = np.concatenate([np.arange(0, r * RB), np.arange((r + 1) * RB, S)])
        perm = np.concatenate([own, rest])
        xb_T = x[b].T  # [D, S]
        kb = k[b]      # [H, DH, P]
        kcp = np.empty((8, 128, P), np.float32)
        for i in range(8):
            kcp[i, 0:64] = kb[2 * i]
            kcp[i, 64:128] = kb[2 * i + 1]
        vb = v[b]      # [H, P, DH]
        vcp = np.ones((H, 128, 8, 65), np.float32)
        # vcp[h, p, tt, :64] = v[b, h, tt*128+p, :]
        vcp[:, :, :, 0:64] = vb.reshape(H, 8, 128, DH).transpose(0, 2, 1, 3)
        im = dict(shared)
        im["xTb"] = np.ascontiguousarray(xb_T[:, perm]).astype(BF16)
        im["xTo"] = np.ascontiguousarray(xb_T[:, own], dtype=np.float32)
        im["kc"] = kcp.astype(BF16)
        im["vc"] = vcp.astype(BF16)
        in_maps.append(im)
    return in_maps


def _run(in_maps, trace=False, tmpdir=None, dbg=False):
    from concourse.bass_utils import run_bass_kernel_spmd

    key = f"nc{int(dbg)}"
    if key not in _CACHE:
        _CACHE[key] = _build_nc(dbg=dbg)
    return run_bass_kernel_spmd(
        _CACHE[key], in_maps, core_ids=list(range(8)), trace=trace, tmpdir=tmpdir
    )


def kernel(**inputs):
    in_maps = _prep_inputs(inputs)
    res = _run(in_maps)
    x_out = np.empty((B, S, D), np.float32)
    new_k = np.empty((B, H, DH, S), np.float32)
    new_v = np.empty((B, H, S, DH), np.float32)
    for c in range(8):
        b, r = c // 4, c % 4
        sl = slice(r * RB, (r + 1) * RB)
        rc = res.results[c]
        x_out[b, sl, :] = rc["xo"]
        new_k[b, :, :, sl] = rc["nk"].reshape(H, DH, RB)
        new_v[b, :, sl, :] = rc["nv"].reshape(RB, H, DH).transpose(1, 0, 2)
    return (x_out, new_k, new_v)


# revision 20
# speedup vs baseline: 1.0341x; 1.0341x over previous
"""Trainium2 Bass kernel for a dense transformer block (B=2, S=1024, D=1024,
H=16, KV-cache P=1024), SPMD over 8 NeuronCores.

Sharding: 8-way split of the (b, s) query rows — cores 0-3 handle batch 0,
cores 4-7 batch 1, each owning 256 rows. The k/v projection (needed for all
rows of the batch by every query) is replicated inside each 4-core batch
group, which removes every cross-core collective. Per-core inputs are
column-permuted so the owned rows always sit in block 0 — one compiled
program serves all 8 cores.

Activations live transposed ([D, S]) on chip so weight matrices feed the
TensorEngine in their natural layout and biases become per-partition vectors.
LayerNorm scale/shift and all projection biases are folded host-side into the
weights/biases. Matmuls run in bf16 (fp32 accumulate); softmax uses
exp(logit - 12) with the denominator produced by an extra ones-column in the
AV matmul's stationary operand.
"""

import numpy as np
import ml_dtypes

BF16 = ml_dtypes.bfloat16

B, S, P, D, H, DH = 2, 1024, 1024, 1024, 16, 64
T = P + S          # total attended keys
F = 4 * D          # MLP hidden
RB = S // 4        # rows owned per core
EPS = 1e-12
ESUB = 12.0        # constant subtracted inside exp (replaces max-subtraction)

_CACHE = {}


def _build_nc(dbg=False):
    import concourse.mybir as mybir
    import concourse.tile as tile
    from concourse import bacc
    from concourse.masks import make_identity

    F32, BF = mybir.dt.float32, mybir.dt.bfloat16
    AF = mybir.ActivationFunctionType
    ADD = mybir.AluOpType.add
    MULT = mybir.AluOpType.mult
    SUBT = mybir.AluOpType.subtract

    nc = bacc.Bacc(None, target_bir_lowering=False, num_devices=8)

    def par(name, shape, dt, out=False):
        return nc.declare_dram_parameter(name, list(shape), dt, isOutput=out)

    xTb = par("xTb", [D, S], BF)            # x[b]^T (permuted cols), bf16
    xTo = par("xTo", [D, RB], F32)          # x[b]^T own cols, f32
    wq = par("wq", [D, D], BF)
    wk = par("wk", [D, D], BF)
    wv = par("wv", [D, D], BF)
    w1 = par("w1", [1, 3 * D], BF)          # column sums of folded wq|wk|wv
    bq = par("bq", [128, 8], F32)           # [p, m] = bias[m*128+p]
    bk = par("bk", [128, 8], F32)
    bvT = par("bvT", [1, D], F32)           # v bias as a row (free-dim layout)
    wp = par("wp", [D, D], BF)
    bp = par("bp", [128, 8], F32)
    wfc = par("wfc", [32, 128, 8, 128], BF)  # [m, p, k, c] tiles
    bfc = par("bfc", [128, 32], F32)
    wfc2 = par("wfc2", [F, D], BF)
    bfc2T = par("bfc2T", [1, D], F32)
    kc = par("kc", [8, 128, P], BF)         # k cache, head pairs packed
    vc = par("vc", [H, 128, 8, 65], BF)     # v cache + ones col, per head
    xo = par("xo", [RB, D], F32, out=True)
    if dbg:
        d_aT = par("d_aT", [128, 8, RB], F32, out=True)
        d_xmid = par("d_xmid", [128, 8, RB], F32, out=True)
        d_h2T = par("d_h2T", [128, 8, RB], F32, out=True)
        d_mT = par("d_mT", [128, 32, RB], F32, out=True)
        d_pav = par("d_pav", [H, 65, RB], F32, out=True)
    nk = par("nk", [D, RB], F32, out=True)
    nv = par("nv", [RB, D], F32, out=True)

    with tile.TileContext(nc) as tc:
        from contextlib import ExitStack

        with (
            tc.tile_pool(name="const", bufs=1) as const,
            tc.tile_pool(name="pers", bufs=1) as pers,
            tc.tile_pool(name="wrow", bufs=8) as wrow,
            tc.tile_pool(name="wfcrow", bufs=2) as wfcrow,
            tc.tile_pool(name="work", bufs=2) as work,
            tc.tile_pool(name="stats", bufs=1) as statsp,
            tc.tile_pool(name="outp", bufs=2) as outp,
            tc.tile_pool(name="pbig", bufs=2, space="PSUM") as pbig,
            tc.tile_pool(name="psmall", bufs=4, space="PSUM") as psmall,
        ):
            # ---- constants ----
            onesb = const.tile([128, 1], BF, tag="onesb", name="onesb")
            nc.vector.memset(onesb[:], 1.0)
            identf = const.tile([128, 128], F32, tag="ident", name="ident")
            make_identity(nc, identf[:])
            identb = const.tile([128, 128], BF, tag="identb", name="identb")
            make_identity(nc, identb[:])
            # cvec columns: 0 nbias(-ESUB), 1 eps, 2:10 bq, 10:18 bk, 18:26 bp,
            # 26:58 bfc
            cvec = const.tile([128, 58], F32, tag="cvec", name="cvec")
            nc.vector.memset(cvec[:, 0:1], -ESUB)
            nc.vector.memset(cvec[:, 1:2], EPS)
            nc.sync.dma_start(out=cvec[:, 2:10], in_=bq[:])
            nc.sync.dma_start(out=cvec[:, 10:18], in_=bk[:])
            nc.sync.dma_start(out=cvec[:, 18:26], in_=bp[:])
            nc.sync.dma_start(out=cvec[:, 26:58], in_=bfc[:])
            nbias = cvec[:, 0:1]
            epst = cvec[:, 1:2]
            bqt = cvec[:, 2:10]
            bkt = cvec[:, 10:18]
            bpt = cvec[:, 18:26]
            bfct = cvec[:, 26:58]

            bvb = const.tile([128, D], F32, tag="bvb", name="bvb")
            bvb_v = bvb[:].rearrange("p (j h d) -> p j h d", j=2, h=8)
            bfc2b = const.tile([128, D], F32, tag="bfc2b", name="bfc2b")

            # ---- persistent activations (consolidated 3D tiles) ----
            xTown = pers.tile([128, 8, RB], F32, tag="xTown", name="xTown")
            qT = pers.tile([128, 8, RB], BF, tag="qT", name="qT")
            aT = pers.tile([128, 8, RB], BF, tag="aT", name="aT")
            xmid = pers.tile([128, 8, RB], F32, tag="xmid", name="xmid")
            xmb = pers.tile([128, 8, RB], BF, tag="xmb", name="xmb")
            h2T = pers.tile([128, 8, RB], BF, tag="h2T", name="h2T")
            mT = pers.tile([128, 32, RB], BF, tag="mT", name="mT")
            xmn = pers.tile([128, 2, D], F32, tag="xmn", name="xmn")  # x_mid natural

            st_a = statsp.tile([1, S], F32, tag="st_a", name="st_a")
            st_b = statsp.tile([1, S], F32, tag="st_b", name="st_b")
            st_c = statsp.tile([1, S], F32, tag="st_c", name="st_c")
            st_d = statsp.tile([1, S], F32, tag="st_d", name="st_d")
            st_rb = statsp.tile([1, S], BF, tag="st_rb", name="st_rb")
            st_sb = statsp.tile([1, S], BF, tag="st_sb", name="st_sb")

            def ln_stats(ps_sx, ps_sx2, n, nfree):
                """returns (rstd_b, negmur_b) [1, nfree] bf16 slices."""
                mean = st_a[:, 0:nfree]
                var = st_b[:, 0:nfree]
                msq = st_c[:, 0:nfree]
                std = st_d[:, 0:nfree]
                rstd = st_c[:, 0:nfree]   # reuses msq slot
                nmr = st_a[:, 0:nfree]    # reuses mean slot
                nc.vector.tensor_scalar_mul(mean, ps_sx[:], 1.0 / n)
                nc.vector.tensor_scalar_mul(var, ps_sx2[:], 1.0 / n)
                nc.vector.tensor_tensor(msq, mean, mean, op=MULT)
                nc.vector.tensor_tensor(var, var, msq, op=SUBT)
                nc.scalar.activation(std, var, AF.Sqrt, bias=epst[0:1, :])
                nc.vector.reciprocal_approx_fast(out=rstd, in_=std)
                nc.vector.tensor_tensor(nmr, mean, rstd, op=MULT)
                nc.vector.tensor_scalar_mul(nmr, nmr, -1.0)
                nc.vector.tensor_copy(st_rb[:, 0:nfree], rstd)
                nc.vector.tensor_copy(st_sb[:, 0:nfree], nmr)
                return st_rb[:, 0:nfree], st_sb[:, 0:nfree]

            scope_c_cm = tc.tile_pool(name="scope_c", bufs=1)
            scope_c = scope_c_cm.__enter__()
            kT = scope_c.tile([128, 8, S], BF, tag="kT", name="kT")
            vS = scope_c.tile([128, 8, H, 65], BF, tag="vS", name="vS")
            scope_b_cm = tc.tile_pool(name="scope_b", bufs=1)
            scope_b = scope_b_cm.__enter__()
            if True:
                xT = scope_b.tile([128, 8, S], BF, tag="xT", name="xT")
                hT = xT  # LayerNorm applied in place
                for i in range(8):
                    eng = nc.sync if i % 2 == 0 else nc.scalar
                    eng.dma_start(
                        out=xT[:, i, :], in_=xTb[i * 128 : (i + 1) * 128, :]
                    )

                # ===== LN1 over full S =====
                ps_sx = pbig.tile([1, S], F32, tag="big", name="ps_sx")
                ps_sx2 = pbig.tile([1, S], F32, tag="big", name="ps_sx2")
                for i in range(8):
                    x2t = work.tile([128, S], BF, tag="x2", name="x2t")
                    nc.vector.tensor_tensor(x2t[:], xT[:, i, :], xT[:, i, :], op=MULT)
                    for j in range(2):
                        sl = slice(j * 512, (j + 1) * 512)
                        nc.tensor.matmul(
                            ps_sx[:, sl], onesb[:], xT[:, i, sl],
                            start=(i == 0), stop=(i == 7),
                        )
                        nc.tensor.matmul(
                            ps_sx2[:, sl], onesb[:], x2t[:, sl],
                            start=(i == 0), stop=(i == 7),
                        )
                # LN1 folded form: r (rstd) + m-tilde rows; no apply pass
                mean1 = st_a[:, 0:S]
                var1 = st_b[:, 0:S]
                msq1 = st_c[:, 0:S]
                std1 = st_d[:, 0:S]
                nc.vector.tensor_scalar_mul(mean1, ps_sx[:], 1.0 / D)
                nc.vector.tensor_scalar_mul(var1, ps_sx2[:], 1.0 / D)
                nc.vector.tensor_tensor(msq1, mean1, mean1, op=MULT)
                nc.vector.tensor_tensor(var1, var1, msq1, op=SUBT)
                nc.scalar.activation(std1, var1, AF.Sqrt, bias=epst[0:1, :])
                nc.vector.reciprocal_approx_fast(out=msq1, in_=std1)
                nc.vector.tensor_copy(st_rb[:, 0:S], msq1)          # r bf16
                nc.vector.tensor_scalar_mul(mean1, mean1, -1.0)
                nc.vector.tensor_copy(st_sb[:, 0:S], mean1)         # m-tilde bf16
                xaug = st_sb
                r1bb = scope_b.tile([128, S], BF, tag="r1bb", name="r1bb")
                nc.gpsimd.partition_broadcast(r1bb[:], st_rb[:, 0:S])
                rcol = scope_b.tile([128, 8], F32, tag="rcol", name="rcol")
                w1r = scope_b.tile([1, 3 * D], BF, tag="w1r", name="w1r")
                nc.sync.dma_start(out=w1r[:], in_=w1[:])

            # ===== q^T [D, RB] (own rows) =====
            wrows = [None] * 8
            for kk in range(8):
                wrows[kk] = wrow.tile([128, D], BF, tag="wrow", name="wqr")
                nc.gpsimd.dma_start(
                    out=wrows[kk][:], in_=wq[kk * 128 : (kk + 1) * 128, :]
                )
            for m in range(8):
                pq = psmall.tile([128, RB], F32, tag="small", name="pq")
                for kk in range(8):
                    nc.tensor.matmul(
                        pq[:], wrows[kk][:, m * 128 : (m + 1) * 128],
                        xT[:, kk, 0:RB], start=(kk == 0), stop=False,
                    )
                nc.tensor.matmul(
                    pq[:], w1r[:, m * 128 : (m + 1) * 128],
                    xaug[:, 0:RB], start=False, stop=True,
                )
                nc.vector.tensor_tensor(
                    qT[:, m, :], pq[:], r1bb[:, 0:RB], op=MULT
                )
                nc.vector.tensor_scalar_add(
                    qT[:, m, :], qT[:, m, :], bqt[:, m : m + 1]
                )

            # ===== k^T [D, S] (all rows; new_k out) =====
            for kk in range(8):
                wrows[kk] = wrow.tile([128, D], BF, tag="wrow", name="wkr")
                nc.scalar.dma_start(
                    out=wrows[kk][:], in_=wk[kk * 128 : (kk + 1) * 128, :]
                )
            for m in range(8):
                for j in range(2):
                    sl = slice(j * 512, (j + 1) * 512)
                    pk = psmall.tile([128, 512], F32, tag="small", name="pk")
                    for kk in range(8):
                        nc.tensor.matmul(
                            pk[:], wrows[kk][:, m * 128 : (m + 1) * 128],
                            xT[:, kk, sl], start=(kk == 0), stop=False,
                        )
                    nc.tensor.matmul(
                        pk[:], w1r[:, D + m * 128 : D + (m + 1) * 128],
                        xaug[:, sl], start=False, stop=True,
                    )
                    nc.vector.tensor_tensor(
                        kT[:, m, sl], pk[:], r1bb[:, sl], op=MULT
                    )
                    nc.vector.tensor_scalar_add(
                        kT[:, m, sl], kT[:, m, sl], bkt[:, m : m + 1]
                    )
                    if j == 0:
                        nkf = outp.tile([128, 512], F32, tag="of", name="nkf")
                        nc.vector.tensor_tensor(
                            nkf[:, 0:RB], pk[:, 0:RB], r1bb[:, 0:RB], op=MULT
                        )
                        nc.vector.tensor_scalar_add(
                            nkf[:, 0:RB], nkf[:, 0:RB], bkt[:, m : m + 1]
                        )
                        nc.sync.dma_start(
                            out=nk[m * 128 : (m + 1) * 128, :], in_=nkf[:, 0:RB]
                        )

            # ===== v [S, D] natural (all rows; new_v out) =====
            for kk in range(8):
                wrows[kk] = wrow.tile([128, D], BF, tag="wrow", name="wvr")
                nc.scalar.dma_start(
                    out=wrows[kk][:], in_=wv[kk * 128 : (kk + 1) * 128, :]
                )
            nc.scalar.dma_start(out=bvb[:], in_=bvT[:].to_broadcast((128, D)))
            nc.vector.memset(vS[:, :, :, 64:65], 1.0)
            for m in range(8):
                ptb = psmall.tile([128, 128], BF, tag="small", name="ptb")
                nc.tensor.transpose(
                    ptb[:], r1bb[:, m * 128 : (m + 1) * 128], identb[:]
                )
                nc.vector.tensor_copy(rcol[:, m : m + 1], ptb[:, 0:1])
            for m in range(8):
                for j in range(2):
                    sl = slice(j * 512, (j + 1) * 512)
                    pv = psmall.tile([128, 512], F32, tag="small", name="pv")
                    for kk in range(8):
                        nc.tensor.matmul(
                            pv[:], xT[:, kk, m * 128 : (m + 1) * 128],
                            wrows[kk][:, sl], start=(kk == 0), stop=False,
                        )
                    nc.tensor.matmul(
                        pv[:], xaug[:, m * 128 : (m + 1) * 128],
                        w1r[:, 2 * D + j * 512 : 2 * D + (j + 1) * 512],
                        start=False, stop=True,
                    )
                    vdst = vS[:, m, j * 8 : (j + 1) * 8, 0:64]
                    nc.vector.tensor_scalar_mul(
                        vdst, pv[:], rcol[:, m : m + 1]
                    )
                    nc.vector.tensor_tensor(vdst, vdst, bvb_v[:, j], op=ADD)
                    if m < 2:
                        nvf = outp.tile([128, 512], F32, tag="of", name="nvf")
                        nc.vector.tensor_scalar_mul(
                            nvf[:], pv[:], rcol[:, m : m + 1]
                        )
                        nc.vector.tensor_tensor(nvf[:], nvf[:], bvb[:, sl], op=ADD)
                        nc.sync.dma_start(
                            out=nv[m * 128 : (m + 1) * 128, sl], in_=nvf[:]
                        )
            scope_b_cm.__exit__(None, None, None)  # xT freed

            nc.gpsimd.dma_start(
                out=xTown[:], in_=xTo[:].rearrange("(i p) s -> p i s", p=128)
            )
            # ===== attention (16 heads) =====
            kch = [None]
            for h in range(H):
                hp, hr = h // 2, (h % 2) * 64
                if h % 2 == 0:
                    kch[0] = scope_c.tile(
                        [128, P], BF, tag="kch", name="kch", bufs=2
                    )
                    nc.gpsimd.dma_start(out=kch[0][:], in_=kc[hp])
                kcht = kch[0]
                vct = scope_c.tile([128, 8, 65], BF, tag="vct", name="vct", bufs=2)
                nc.gpsimd.dma_start(out=vct[:], in_=vc[h])

                pav = psmall.tile([65, RB], F32, tag="small", name="pav")
                for g in range(4):  # 4 groups of 4 t-tiles
                    pl = pbig.tile([128, 4 * RB], F32, tag="big", name="pl")
                    for tl in range(4):
                        tt = g * 4 + tl
                        if tt < 8:
                            lh = kcht[hr : hr + 64, tt * 128 : (tt + 1) * 128]
                        else:
                            lh = kT[hr : hr + 64, hp, (tt - 8) * 128 : (tt - 7) * 128]
                        nc.tensor.matmul(
                            pl[:, tl * RB : (tl + 1) * RB], lh,
                            qT[hr : hr + 64, hp, :], start=True, stop=True,
                        )
                    wex = scope_c.tile(
                        [128, 4 * RB], BF, tag="wex", name="wex", bufs=2
                    )
                    nc.scalar.activation(wex[:], pl[:], AF.Exp, bias=nbias)
                    for tl in range(4):
                        tt = g * 4 + tl
                        vv = vct[:, tt, :] if tt < 8 else vS[:, tt - 8, h, :]
                        nc.tensor.matmul(
                            pav[:], vv, wex[:, tl * RB : (tl + 1) * RB],
                            start=(tt == 0), stop=(tt == 15),
                        )
                if dbg:
                    dpav = outp.tile([65, RB], F32, tag="of", name="dbgpav")
                    nc.vector.tensor_copy(dpav[:], pav[:])
                    nc.sync.dma_start(out=d_pav[h], in_=dpav[:])
                rec = scope_c.tile([1, 2, RB], F32, tag="rec", name="rec", bufs=1)
                nc.vector.tensor_copy(rec[:, 0, :], pav[64:65, :])
                nc.vector.reciprocal_approx_fast(out=rec[:, 1, :], in_=rec[:, 0, :])
                rb64 = scope_c.tile([64, RB], F32, tag="rb64", name="rb64", bufs=2)
                nc.gpsimd.partition_broadcast(rb64[:], rec[:, 1, :])
                nc.vector.tensor_tensor(
                    aT[hr : hr + 64, hp, :], pav[0:64, :], rb64[:], op=MULT
                )

            if dbg:
                for i in range(8):
                    dt_ = outp.tile([128, RB], F32, tag="of", name="dbg1")
                    nc.vector.tensor_copy(dt_[:], aT[:, i, :])
                    nc.sync.dma_start(out=d_aT[:, i, :], in_=dt_[:])

            # ===== proj + residual -> x_mid^T [D, RB] f32 =====
            for kk in range(8):
                wrows[kk] = wrow.tile([128, D], BF, tag="wrow", name="wpr")
                nc.scalar.dma_start(
                    out=wrows[kk][:], in_=wp[kk * 128 : (kk + 1) * 128, :]
                )
            for m in range(8):
                pp = psmall.tile([128, RB], F32, tag="small", name="pp")
                for kk in range(8):
                    nc.tensor.matmul(
                        pp[:], wrows[kk][:, m * 128 : (m + 1) * 128],
                        aT[:, kk, :], start=(kk == 0), stop=(kk == 7),
                    )
                nc.vector.tensor_scalar_add(xmid[:, m, :], pp[:], bpt[:, m : m + 1])
                nc.vector.tensor_tensor(
                    xmid[:, m, :], xmid[:, m, :], xTown[:, m, :], op=ADD
                )
            scope_c_cm.__exit__(None, None, None)  # kT / vS / attn work freed

            if dbg:
                for i in range(8):
                    dt_ = outp.tile([128, RB], F32, tag="of", name="dbg2")
                    nc.vector.tensor_copy(dt_[:], xmid[:, i, :])
                    nc.sync.dma_start(out=d_xmid[:, i, :], in_=dt_[:])

            # ===== LN2 (over own RB cols) =====
            ps2_sx = pbig.tile([1, RB], F32, tag="big", name="ps2_sx")
            ps2_sx2 = pbig.tile([1, RB], F32, tag="big", name="ps2_sx2")
            for i in range(8):
                nc.vector.tensor_copy(xmb[:, i, :], xmid[:, i, :])
                x2t = work.tile([128, RB], BF, tag="x2", name="x2s")
                nc.vector.tensor_tensor(x2t[:], xmb[:, i, :], xmb[:, i, :], op=MULT)
                nc.tensor.matmul(
                    ps2_sx[:], onesb[:], xmb[:, i, :], start=(i == 0), stop=(i == 7)
                )
                nc.tensor.matmul(
                    ps2_sx2[:], onesb[:], x2t[:], start=(i == 0), stop=(i == 7)
                )
            r2b, s2b = ln_stats(ps2_sx, ps2_sx2, D, RB)
            r2bb = work.tile([128, RB], BF, tag="r2bb", name="r2bb")
            nc.gpsimd.partition_broadcast(r2bb[:], r2b)
            s2bb = work.tile([128, RB], BF, tag="s2bb", name="s2bb")
            nc.gpsimd.partition_broadcast(s2bb[:], s2b)
            for i in range(8):
                nc.vector.tensor_tensor(h2T[:, i, :], xmb[:, i, :], r2bb[:], op=MULT)
                nc.vector.tensor_tensor(h2T[:, i, :], h2T[:, i, :], s2bb[:], op=ADD)

            if dbg:
                for i in range(8):
                    dt_ = outp.tile([128, RB], F32, tag="of", name="dbg3")
                    nc.vector.tensor_copy(dt_[:], h2T[:, i, :])
                    nc.sync.dma_start(out=d_h2T[:, i, :], in_=dt_[:])

            # ===== transpose x_mid -> natural [RB, D] (for final residual) ==
            for m in range(8):
                for j in range(2):
                    ptr = pbig.tile([128, 128], F32, tag="big", name="ptr")
                    nc.tensor.transpose(
                        ptr[:], xmid[:, m, j * 128 : (j + 1) * 128], identf[:]
                    )
                    nc.vector.tensor_copy(
                        xmn[:, j, m * 128 : (m + 1) * 128], ptr[:]
                    )

            # ===== MLP fc + gelu -> m^T [F, RB] bf16 =====
            for m in range(32):
                wfcm = wfcrow.tile([128, 8, 128], BF, tag="wfcm", name="wfcm", bufs=6)
                nc.sync.dma_start(out=wfcm[:], in_=wfc[m])  # [p, k, c] both sides
                pf = psmall.tile([128, RB], F32, tag="small", name="pf")
                for kk in range(8):
                    nc.tensor.matmul(
                        pf[:], wfcm[:, kk, :],
                        h2T[:, kk, :], start=(kk == 0), stop=(kk == 7),
                    )
                nc.scalar.activation(
                    mT[:, m, :], pf[:], AF.Gelu, bias=bfct[:, m : m + 1]
                )

            if dbg:
                for i in range(32):
                    dt_ = outp.tile([128, RB], F32, tag="of", name="dbg4")
                    nc.vector.tensor_copy(dt_[:], mT[:, i, :])
                    nc.sync.dma_start(out=d_mT[:, i, :], in_=dt_[:])

            nc.scalar.dma_start(
                out=bfc2b[:], in_=bfc2T[:].to_broadcast((128, D))
            )
            scope_d_cm = tc.tile_pool(name="scope_d", bufs=1)
            scope_d = scope_d_cm.__enter__()
            # ===== MLP fc2 (natural orientation) + residual -> x_out ========
            pf2 = [
                pbig.tile([128, D], F32, tag="big", name="pf2a"),
                pbig.tile([128, D], F32, tag="big", name="pf2b"),
            ]
            for kk in range(32):
                wr2 = scope_d.tile([128, D], BF, tag="wfc2r", name="wfc2r", bufs=16)
                eng = nc.sync if kk % 2 == 0 else nc.scalar
                eng.dma_start(
                    out=wr2[:], in_=wfc2[kk * 128 : (kk + 1) * 128, :]
                )
                for m in range(2):
                    for j in range(2):
                        sl = slice(j * 512, (j + 1) * 512)
                        nc.tensor.matmul(
                            pf2[m][:, sl],
                            mT[:, kk, m * 128 : (m + 1) * 128],
                            wr2[:, sl], start=(kk == 0), stop=(kk == 31),
                        )
            for m in range(2):
                for j in range(2):
                    sl = slice(j * 512, (j + 1) * 512)
                    xon = outp.tile([128, 512], F32, tag="of", name="xon")
                    nc.vector.tensor_tensor(
                        xon[:], pf2[m][:, sl], bfc2b[:, sl], op=ADD
                    )
                    nc.vector.tensor_tensor(
                        xon[:], xon[:], xmn[:, m, sl], op=ADD
                    )
                    nc.sync.dma_start(
                        out=xo[m * 128 : (m + 1) * 128, sl], in_=xon[:]
                    )
            scope_d_cm.__exit__(None, None, None)

    nc.compile()
    return nc


def _prep_inputs(inputs):
    x = np.asarray(inputs["x"], np.float32)
    k = np.asarray(inputs["k"], np.float32)
    v = np.asarray(inputs["v"], np.float32)
    ln1_w = np.asarray(inputs["ln1_w"], np.float32)
    ln1_b = np.asarray(inputs["ln1_b"], np.float32)
    ln2_w = np.asarray(inputs["ln2_w"], np.float32)
    ln2_b = np.asarray(inputs["ln2_b"], np.float32)
    w_attn = np.asarray(inputs["w_attn"], np.float32)
    b_attn = np.asarray(inputs["b_attn"], np.float32)
    w_proj = np.asarray(inputs["w_proj"], np.float32)
    b_proj = np.asarray(inputs["b_proj"], np.float32)
    w_fc = np.asarray(inputs["w_fc"], np.float32)
    b_fc = np.asarray(inputs["b_fc"], np.float32)
    w_fc2 = np.asarray(inputs["w_fc2"], np.float32)
    b_fc2 = np.asarray(inputs["b_fc2"], np.float32)

    W1 = ln1_w[:, None] * w_attn
    bqkv = b_attn + ln1_b @ w_attn
    Wfc = ln2_w[:, None] * w_fc
    bfc_f = b_fc + ln2_b @ w_fc

    def colmaj(b):  # [n*128] -> [128, n] with [p, m] = b[m*128+p]
        return np.ascontiguousarray(b.reshape(-1, 128).T, dtype=np.float32)

    shared = {
        "wq": np.ascontiguousarray(W1[:, :D]).astype(BF16),
        "wk": np.ascontiguousarray(W1[:, D : 2 * D]).astype(BF16),
        "wv": np.ascontiguousarray(W1[:, 2 * D :]).astype(BF16),
        "w1": np.ascontiguousarray(
            np.concatenate(
                [W1[:, :D].sum(0), W1[:, D : 2 * D].sum(0), W1[:, 2 * D :].sum(0)]
            ).reshape(1, 3 * D)
        ).astype(BF16),
        "bq": colmaj(bqkv[:D]),
        "bk": colmaj(bqkv[D : 2 * D]),
        "bvT": np.ascontiguousarray(bqkv[2 * D :].reshape(1, D), dtype=np.float32),
        "wp": w_proj.astype(BF16),
        "bp": colmaj(b_proj),
        "wfc": np.ascontiguousarray(
            Wfc.reshape(8, 128, 32, 128).transpose(2, 1, 0, 3)
        ).astype(BF16),
        "bfc": colmaj(bfc_f),
        "wfc2": w_fc2.astype(BF16),
        "bfc2T": np.ascontiguousarray(b_fc2.reshape(1, D), dtype=np.float32),
    }

    in_maps = []
    for c in range(8):
        b, r = c // 4, c % 4
        own = np.arange(r * RB, (r + 1) * RB)
        rest = np.concatenate([np.arange(0, r * RB), np.arange((r + 1) * RB, S)])
        perm = np.concatenate([own, rest])
        xb_T = x[b].T  # [D, S]
        kb = k[b]      # [H, DH, P]
        kcp = np.empty((8, 128, P), np.float32)
        for i in range(8):
            kcp[i, 0:64] = kb[2 * i]
            kcp[i, 64:128] = kb[2 * i + 1]
        vb = v[b]      # [H, P, DH]
        vcp = np.ones((H, 128, 8, 65), np.float32)
        # vcp[h, p, tt, :64] = v[b, h, tt*128+p, :]
        vcp[:, :, :, 0:64] = vb.reshape(H, 8, 128, DH).transpose(0, 2, 1, 3)
        im = dict(shared)
        im["xTb"] = np.ascontiguousarray(xb_T[:, perm]).astype(BF16)
        im["xTo"] = np.ascontiguousarray(xb_T[:, own], dtype=np.float32)
        im["kc"] = kcp.astype(BF16)
        im["vc"] = vcp.astype(BF16)
        in_maps.append(im)
    return in_maps


def _run(in_maps, trace=False, tmpdir=None, dbg=False):
    from concourse.bass_utils import run_bass_kernel_spmd

    key = f"nc{int(dbg)}"
    if key not in _CACHE:
        _CACHE[key] = _build_nc(dbg=dbg)
    return run_bass_kernel_spmd(
        _CACHE[key], in_maps, core_ids=list(range(8)), trace=trace, tmpdir=tmpdir
    )


def kernel(**inputs):
    in_maps = _prep_inputs(inputs)
    res = _run(in_maps)
    x_out = np.empty((B, S, D), np.float32)
    new_k = np.empty((B, H, DH, S), np.float32)
    new_v = np.empty((B, H, S, DH), np.float32)
    for c in range(8):
        b, r = c // 4, c % 4
        sl = slice(r * RB, (r + 1) * RB)
        rc = res.results[c]
        x_out[b, sl, :] = rc["xo"]
        new_k[b, :, :, sl] = rc["nk"].reshape(H, DH, RB)
        new_v[b, :, sl, :] = rc["nv"].reshape(RB, H, DH).transpose(1, 0, 2)
    return (x_out, new_k, new_v)
